# revision 2
# baseline (speedup 1.0000x reference)
"""LoRA linear layer on 8 Trainium2 NeuronCores.

Computes y = x @ W^T + b + 2.0 * (x @ A^T) @ B^T for
x:[4,4096,1024], W:[1024,1024], b:[1024], A:[16,1024], B:[1024,16].

Host side folds the LoRA update into the weight (W_eff = W + 2*B@A, an exact
algebraic identity), so the device kernel is a single GEMM + bias. Sharding is
data-parallel over the 16384 tokens: each of the 8 cores computes a
[2048, 1024] output slice with replicated weights.

Device kernel (per core): y_c[m,o] = sum_d xT_c[d,m] * WeffT[d,o] + b[o]

v2 design, driven by trace analysis of the v1 baseline (72.9 us):
  - The 16 hw DMA channels were only ~41% busy; the real DMA cost is the
    ~600 ns DIRECT2D descriptor-gen per dma_start on the issuing sequencer
    plus one completion semaphore per transfer (the postamble drains every
    semaphore serially at ~100 ns each). So v2 collapses ~120 dma_starts
    into ~25 big multi-run-AP transfers: x loads one dma_start per 512-token
    super-chunk (8 KiB/partition), W in 4 loads, stores batched per m-tile.
  - Only sync + scalar (the two hardware-DGE queues) issue DMAs; gpsimd's
    slow software-DGE queue is not used at all.
  - Warmup matmuls start immediately at user-code start to pull the HAM
    high-clock window (1.2 -> 2.4 GHz, ~4.2 us after sustained PE activity
    begins) as early as possible; real matmuls begin ~2.4 us in, at low
    clock, which still beats idling until the window opens.
  - sc0 runs ko-outer (all 8 PSUM groups accumulate together, so each
    arriving x/W slice is consumed immediately during the DMA ramp);
    sc1..3 run mt-outer (2 PSUM groups in flight, so the single DVE never
    gates PSUM-bank recycling and output flows out per m-tile).
  - Tail: the final m-tile computes h0 as one 512-col group, then h1 as two
    256-col groups, so the last serial evict+store chain is minimal.
"""

import os

import numpy as np
from ml_dtypes import bfloat16

import concourse.mybir as mybir
import concourse.tile as tile
from concourse import bacc
from concourse.bass_utils import run_bass_kernel_spmd

N_CORES = 8
P = 128
D = 1024  # in_features (contraction)
O = 1024  # out_features
M_TOTAL = 4 * 4096  # tokens
M = M_TOTAL // N_CORES  # tokens per core
KO = D // P  # k-subtiles
SC = 512  # m super-chunk
MPC = SC // P  # m-tiles per super-chunk
SCALING = 2.0

# Set by test harnesses to capture profiling info; harmless otherwise.
TRACE = False
LAST_RESULT = None

_NC_CACHE = None


def _build_nc():
    f32 = mybir.dt.float32
    bf16 = mybir.dt.bfloat16

    nc = bacc.Bacc("TRN2", debug=False)
    xT = nc.dram_tensor("xT", [D, M], bf16, kind="ExternalInput")
    wT = nc.dram_tensor("wT", [D, O], bf16, kind="ExternalInput")
    bias = nc.dram_tensor("bias", [P, O], f32, kind="ExternalInput")
    y = nc.dram_tensor("y", [M, O], bf16, kind="ExternalOutput")

    xT_v = xT[:].rearrange("(ko p) m -> p ko m", p=P)  # [128, 8, 2048]
    wT_v = wT[:].rearrange("(ko p) o -> p ko o", p=P)  # [128, 8, 1024]
    y_v = y[:].rearrange("(mt p) o -> p mt o", p=P)  # [128, 16, 1024]

    n_sc = M // SC  # 4
    with tile.TileContext(nc) as tc:
        with (
            tc.tile_pool(name="wpool", bufs=1) as wpool,
            tc.tile_pool(name="xpool", bufs=1) as xpool,
            tc.tile_pool(name="opool", bufs=3) as opool,
            tc.tile_pool(name="psum", bufs=8, space="PSUM") as psum,
        ):
            # --- loads -------------------------------------------------
            # sync: all of x. First ko of sc0 goes alone (small, so the
            # first real matmul can start ~2.4 us in); the rest of sc0 as
            # one transfer; sc1..3 as one 1 MiB transfer each.
            x00 = xpool.tile([P, SC], bf16, tag="x00")
            nc.sync.dma_start(x00[:], xT_v[:, 0, 0:SC])
            x0r = xpool.tile([P, (KO - 1) * SC], bf16, tag="x0r")
            nc.sync.dma_start(
                x0r[:].rearrange("p (ko m) -> p ko m", ko=KO - 1),
                xT_v[:, 1:KO, 0:SC],
            )
            xs = {}
            for sc in range(1, n_sc):
                t = xpool.tile([P, KO * SC], bf16, tag=f"xs{sc}")
                nc.sync.dma_start(
                    t[:].rearrange("p (ko m) -> p ko m", ko=KO),
                    xT_v[:, :, sc * SC : (sc + 1) * SC],
                )
                xs[sc] = t

            # scalar: W (ko0 split in halves for the earliest possible
            # first matmul, ko1 alone, ko2..7 as one transfer) + bias.
            w0h = []
            for half in range(2):
                t = wpool.tile([P, 512], bf16, tag=f"w0_{half}")
                nc.scalar.dma_start(t[:], wT_v[:, 0, half * 512 : (half + 1) * 512])
                w0h.append(t)
            w1 = wpool.tile([P, O], bf16, tag="w1")
            nc.scalar.dma_start(w1[:], wT_v[:, 1, :])
            w27 = wpool.tile([P, (KO - 2) * O], bf16, tag="w27")
            nc.scalar.dma_start(
                w27[:].rearrange("p (ko o) -> p ko o", ko=KO - 2),
                wT_v[:, 2:KO, :],
            )
            bt = wpool.tile([P, O], f32, tag="bias")
            nc.scalar.dma_start(bt[:], bias[:])

            # --- warmups ----------------------------------------------
            # Start PE activity immediately so the HAM clock window opens
            # as early as possible; sized to end right as the first real
            # matmul's data lands.
            n_warm = int(os.environ.get("KWARM", "18"))
            if n_warm:
                zt = wpool.tile([P, P], bf16, tag="warm")
                nc.vector.memset(zt[:], 0.0)
                wps = psum.tile([P, P], f32, tag="ps", name="wps")
                for _ in range(n_warm):
                    nc.tensor.matmul(wps[:], zt[:], zt[:], start=True, stop=True)

            def x_slice(sc, ko, mt_i):
                if sc == 0:
                    if ko == 0:
                        return x00[:, mt_i * P : (mt_i + 1) * P]
                    return x0r[:, (ko - 1) * SC + mt_i * P : (ko - 1) * SC + (mt_i + 1) * P]
                return xs[sc][:, ko * SC + mt_i * P : ko * SC + (mt_i + 1) * P]

            def w_op(ko, half):
                if ko == 0:
                    return w0h[half][:]
                if ko == 1:
                    return w1[:, half * 512 : (half + 1) * 512]
                return w27[:, (ko - 2) * O + half * 512 : (ko - 2) * O + (half + 1) * 512]

            def evict(ps, out_ap, lo, hi):
                nc.vector.tensor_tensor(
                    out_ap, ps[:], bt[:, lo:hi], mybir.AluOpType.add
                )

            # --- sc0: ko-outer, 8 concurrent PSUM groups ---------------
            # Consumes each x/W slice the moment it lands during the ramp.
            ot0 = opool.tile([P, MPC * O], bf16, tag="ot0")
            pss = [
                [psum.tile([P, 512], f32, tag="ps", name=f"ps0_{i}_{h}") for h in range(2)]
                for i in range(MPC)
            ]
            for ko in range(KO):
                last = ko == KO - 1
                for half in range(2):
                    for mt_i in range(MPC):
                        nc.tensor.matmul(
                            pss[mt_i][half][:],
                            x_slice(0, ko, mt_i),
                            w_op(ko, half),
                            start=ko == 0,
                            stop=last,
                        )
                        if last:
                            lo = half * 512
                            evict(
                                pss[mt_i][half],
                                ot0[:, mt_i * O + lo : mt_i * O + lo + 512],
                                lo,
                                lo + 512,
                            )
            nc.sync.dma_start(
                y_v[:, 0:MPC, :],
                ot0[:].rearrange("p (mt o) -> p mt o", mt=MPC),
            )

            # --- sc1..3: mt-outer, 2 PSUM groups in flight -------------
            # Store engine: sync for sc1/2 (idle after the x loads),
            # scalar for sc3 (idle after the W loads).
            for sc in range(1, n_sc):
                st_eng = nc.sync if sc < n_sc - 1 else nc.scalar
                for mt_i in range(MPC):
                    mt = sc * MPC + mt_i
                    final = sc == n_sc - 1 and mt_i == MPC - 1
                    ot = opool.tile([P, O], bf16, tag="otf", name=f"ot{sc}_{mt_i}")
                    if not final:
                        ph = [
                            psum.tile([P, 512], f32, tag="ps", name=f"p{sc}_{mt_i}_{h}")
                            for h in range(2)
                        ]
                        for ko in range(KO):
                            for half in range(2):
                                nc.tensor.matmul(
                                    ph[half][:],
                                    x_slice(sc, ko, mt_i),
                                    w_op(ko, half),
                                    start=ko == 0,
                                    stop=ko == KO - 1,
                                )
                        for half in range(2):
                            lo = half * 512
                            evict(ph[half], ot[:, lo : lo + 512], lo, lo + 512)
                        st_eng.dma_start(y_v[:, mt, :], ot[:])
                    else:
                        # Final m-tile: smallest possible serial tail.
                        # h0 as one 512 group, stored while h1 computes;
                        # h1 as two 256 groups on alternating store queues.
                        ps = psum.tile([P, 512], f32, tag="ps", name="pf")
                        for ko in range(KO):
                            nc.tensor.matmul(
                                ps[:],
                                x_slice(sc, ko, mt_i),
                                w_op(ko, 0),
                                start=ko == 0,
                                stop=ko == KO - 1,
                            )
                        evict(ps, ot[:, 0:512], 0, 512)
                        nc.sync.dma_start(y_v[:, mt, 0:512], ot[:, 0:512])
                        for q in range(2):
                            lo = 512 + q * 256
                            psq = psum.tile([P, 256], f32, tag="ps", name=f"pfq{q}")
                            for ko in range(KO):
                                nc.tensor.matmul(
                                    psq[:],
                                    x_slice(sc, ko, mt_i),
                                    w_op(ko, 1)[:, q * 256 : q * 256 + 256],
                                    start=ko == 0,
                                    stop=ko == KO - 1,
                                )
                            evict(psq, ot[:, lo : lo + 256], lo, lo + 256)
                            eng = nc.scalar if q == 0 else nc.sync
                            eng.dma_start(y_v[:, mt, lo : lo + 256], ot[:, lo : lo + 256])

    nc.compile()
    return nc


def _get_nc():
    global _NC_CACHE
    if _NC_CACHE is None:
        _NC_CACHE = _build_nc()
    return _NC_CACHE


def kernel(x, W, b, A, B):
    global LAST_RESULT
    x = np.ascontiguousarray(np.asarray(x, dtype=np.float32))
    W = np.asarray(W, dtype=np.float32)
    b = np.asarray(b, dtype=np.float32)
    A = np.asarray(A, dtype=np.float32)
    B = np.asarray(B, dtype=np.float32)
    assert x.shape == (4, 4096, D) and W.shape == (O, D)
    assert b.shape == (O,) and A.shape[1] == D and B.shape[0] == O

    # Fold the LoRA update into the weight: x@W^T + s*(x@A^T)@B^T = x@(W + s*B@A)^T
    Weff = (
        W.astype(np.float64) + SCALING * (B.astype(np.float64) @ A.astype(np.float64))
    ).astype(np.float32)
    WeffT = np.ascontiguousarray(Weff.T).astype(bfloat16)  # [D, O]
    bias_rep = np.ascontiguousarray(np.broadcast_to(b[None, :], (P, O)))

    xr = x.reshape(M_TOTAL, D)
    in_maps = []
    for c in range(N_CORES):
        xTc = np.ascontiguousarray(xr[c * M : (c + 1) * M].T).astype(bfloat16)  # [D, M]
        in_maps.append({"xT": xTc, "wT": WeffT, "bias": bias_rep})

    nc = _get_nc()
    res = run_bass_kernel_spmd(
        nc, in_maps, core_ids=list(range(N_CORES)), trace=TRACE
    )
    LAST_RESULT = res

    out = np.concatenate(
        [res.results[c]["y"].astype(np.float32) for c in range(N_CORES)], axis=0
    )
    return out.reshape(x.shape[0], x.shape[1], O)


# revision 5
# speedup vs baseline: 1.1438x; 1.1438x over previous
"""LoRA linear layer on 8 Trainium2 NeuronCores.

Computes y = x @ W^T + b + 2.0 * (x @ A^T) @ B^T for
x:[4,4096,1024], W:[1024,1024], b:[1024], A:[16,1024], B:[1024,16].

Host side folds the LoRA update into the weight (W_eff = W + 2*B@A, an exact
algebraic identity), so the device kernel is a single GEMM + bias. Sharding is
data-parallel over the 16384 tokens: each of the 8 cores computes a
[2048, 1024] output slice with replicated weights.

Device kernel (per core): y_c[m,o] = sum_d xT_c[d,m] * WeffT[d,o] + b[o]

v2 design, driven by trace analysis of the v1 baseline (72.9 us):
  - The 16 hw DMA channels were only ~41% busy; the real DMA cost is the
    ~600 ns DIRECT2D descriptor-gen per dma_start on the issuing sequencer
    plus one completion semaphore per transfer (the postamble drains every
    semaphore serially at ~100 ns each). So v2 collapses ~120 dma_starts
    into ~25 big multi-run-AP transfers: x loads one dma_start per 512-token
    super-chunk (8 KiB/partition), W in 4 loads, stores batched per m-tile.
  - Only sync + scalar (the two hardware-DGE queues) issue DMAs; gpsimd's
    slow software-DGE queue is not used at all.
  - Warmup matmuls start immediately at user-code start to pull the HAM
    high-clock window (1.2 -> 2.4 GHz, ~4.2 us after sustained PE activity
    begins) as early as possible; real matmuls begin ~2.4 us in, at low
    clock, which still beats idling until the window opens.
  - sc0 runs ko-outer (all 8 PSUM groups accumulate together, so each
    arriving x/W slice is consumed immediately during the DMA ramp);
    sc1..3 run mt-outer (2 PSUM groups in flight, so the single DVE never
    gates PSUM-bank recycling and output flows out per m-tile).
  - Tail: the final m-tile computes h0 as one 512-col group, then h1 as two
    256-col groups, so the last serial evict+store chain is minimal.
"""

import os

import numpy as np
from ml_dtypes import bfloat16

import concourse.mybir as mybir
import concourse.tile as tile
from concourse import bacc
from concourse.bass_utils import run_bass_kernel_spmd

N_CORES = 8
P = 128
D = 1024  # in_features (contraction)
O = 1024  # out_features
M_TOTAL = 4 * 4096  # tokens
M = M_TOTAL // N_CORES  # tokens per core
KO = D // P  # k-subtiles
SC = 512  # m super-chunk
MPC = SC // P  # m-tiles per super-chunk
SCALING = 2.0

# Set by test harnesses to capture profiling info; harmless otherwise.
TRACE = False
LAST_RESULT = None

_NC_CACHE = None


def _build_nc():
    f32 = mybir.dt.float32
    bf16 = mybir.dt.bfloat16

    nc = bacc.Bacc("TRN2", debug=False)
    xT = nc.dram_tensor("xT", [D, M], bf16, kind="ExternalInput")
    wT = nc.dram_tensor("wT", [D, O], bf16, kind="ExternalInput")
    bias = nc.dram_tensor("bias", [P, O], f32, kind="ExternalInput")
    y = nc.dram_tensor("y", [M, O], bf16, kind="ExternalOutput")

    xT_v = xT[:].rearrange("(ko p) m -> p ko m", p=P)  # [128, 8, 2048]
    wT_v = wT[:].rearrange("(ko p) o -> p ko o", p=P)  # [128, 8, 1024]
    y_v = y[:].rearrange("(mt p) o -> p mt o", p=P)  # [128, 16, 1024]

    n_sc = M // SC  # 4
    with tile.TileContext(nc) as tc:
        with (
            tc.tile_pool(name="wpool", bufs=1) as wpool,
            tc.tile_pool(name="xpool", bufs=1) as xpool,
            tc.tile_pool(name="opool", bufs=3) as opool,
            tc.tile_pool(name="psum", bufs=8, space="PSUM") as psum,
        ):
            # --- loads -------------------------------------------------
            # Transfers on one engine's ring serialize, and the two rings
            # share the 16 hw channels, so both rings must deliver in
            # consumption-deadline order: per-ko granules for sc0's x
            # (sync ring) and for W (scalar ring) arrive in lockstep with
            # the PE's ko cadence; the big sc1..3 x prefetches queue on
            # sync strictly AFTER sc0's granules so they never starve the
            # critical W stream (the v2 mistake, -5.4 us stall).
            x0k = []
            for ko in range(KO):
                t = xpool.tile([P, SC], bf16, tag=f"x0_{ko}")
                nc.sync.dma_start(t[:], xT_v[:, ko, 0:SC])
                x0k.append(t)
            xs = {}
            for sc in range(1, n_sc):
                t = xpool.tile([P, KO * SC], bf16, tag=f"xs{sc}")
                nc.sync.dma_start(
                    t[:].rearrange("p (ko m) -> p ko m", ko=KO),
                    xT_v[:, :, sc * SC : (sc + 1) * SC],
                )
                xs[sc] = t

            # scalar: W per-ko (ko0 split in halves for the earliest
            # possible first matmul), then bias last (first needed only
            # at the first eviction, ~20 us in).
            w0h = []
            for half in range(2):
                t = wpool.tile([P, 512], bf16, tag=f"w0_{half}")
                nc.scalar.dma_start(t[:], wT_v[:, 0, half * 512 : (half + 1) * 512])
                w0h.append(t)
            wt = [None] * KO
            for ko in range(1, KO):
                t = wpool.tile([P, O], bf16, tag=f"w{ko}")
                nc.scalar.dma_start(t[:], wT_v[:, ko, :])
                wt[ko] = t
            bt = wpool.tile([P, O], f32, tag="bias")
            nc.scalar.dma_start(bt[:], bias[:])

            # --- warmups ----------------------------------------------
            # Start PE activity immediately so the HAM clock window opens
            # as early as possible; sized to end right as the first real
            # matmul's data lands.
            n_warm = int(os.environ.get("KWARM", "26"))
            if n_warm:
                zt = wpool.tile([P, P], bf16, tag="warm")
                nc.vector.memset(zt[:], 0.0)
                wps = psum.tile([P, P], f32, tag="ps", name="wps")
                for _ in range(n_warm):
                    nc.tensor.matmul(wps[:], zt[:], zt[:], start=True, stop=True)

            def x_slice(sc, ko, mt_i):
                if sc == 0:
                    return x0k[ko][:, mt_i * P : (mt_i + 1) * P]
                return xs[sc][:, ko * SC + mt_i * P : ko * SC + (mt_i + 1) * P]

            def w_op(ko, half):
                if ko == 0:
                    return w0h[half][:]
                return wt[ko][:, half * 512 : (half + 1) * 512]

            def evict(ps, out_ap, lo, hi):
                nc.vector.tensor_tensor(
                    out_ap, ps[:], bt[:, lo:hi], mybir.AluOpType.add
                )

            # --- sc0: ko-outer, 8 concurrent PSUM groups ---------------
            # Consumes each x/W slice the moment it lands during the ramp.
            ot0 = opool.tile([P, MPC * O], bf16, tag="ot0")
            pss = [
                [psum.tile([P, 512], f32, tag="ps", name=f"ps0_{i}_{h}") for h in range(2)]
                for i in range(MPC)
            ]
            for ko in range(KO):
                last = ko == KO - 1
                for half in range(2):
                    for mt_i in range(MPC):
                        nc.tensor.matmul(
                            pss[mt_i][half][:],
                            x_slice(0, ko, mt_i),
                            w_op(ko, half),
                            start=ko == 0,
                            stop=last,
                        )
                        if last:
                            lo = half * 512
                            evict(
                                pss[mt_i][half],
                                ot0[:, mt_i * O + lo : mt_i * O + lo + 512],
                                lo,
                                lo + 512,
                            )
            nc.sync.dma_start(
                y_v[:, 0:MPC, :],
                ot0[:].rearrange("p (mt o) -> p mt o", mt=MPC),
            )

            # --- sc1..3: mt-outer, 2 PSUM groups in flight -------------
            # Store engine: sync for sc1/2 (idle after the x loads),
            # scalar for sc3 (idle after the W loads).
            for sc in range(1, n_sc):
                st_eng = nc.sync if sc < n_sc - 1 else nc.scalar
                for mt_i in range(MPC):
                    mt = sc * MPC + mt_i
                    final = sc == n_sc - 1 and mt_i == MPC - 1
                    ot = opool.tile([P, O], bf16, tag="otf", name=f"ot{sc}_{mt_i}")
                    if not final:
                        ph = [
                            psum.tile([P, 512], f32, tag="ps", name=f"p{sc}_{mt_i}_{h}")
                            for h in range(2)
                        ]
                        for ko in range(KO):
                            for half in range(2):
                                nc.tensor.matmul(
                                    ph[half][:],
                                    x_slice(sc, ko, mt_i),
                                    w_op(ko, half),
                                    start=ko == 0,
                                    stop=ko == KO - 1,
                                )
                        for half in range(2):
                            lo = half * 512
                            evict(ph[half], ot[:, lo : lo + 512], lo, lo + 512)
                        st_eng.dma_start(y_v[:, mt, :], ot[:])
                    else:
                        # Final m-tile: smallest possible serial tail.
                        # h0 as one 512 group, stored while h1 computes;
                        # h1 as two 256 groups on alternating store queues.
                        ps = psum.tile([P, 512], f32, tag="ps", name="pf")
                        for ko in range(KO):
                            nc.tensor.matmul(
                                ps[:],
                                x_slice(sc, ko, mt_i),
                                w_op(ko, 0),
                                start=ko == 0,
                                stop=ko == KO - 1,
                            )
                        evict(ps, ot[:, 0:512], 0, 512)
                        nc.sync.dma_start(y_v[:, mt, 0:512], ot[:, 0:512])
                        for q in range(2):
                            lo = 512 + q * 256
                            psq = psum.tile([P, 256], f32, tag="ps", name=f"pfq{q}")
                            for ko in range(KO):
                                nc.tensor.matmul(
                                    psq[:],
                                    x_slice(sc, ko, mt_i),
                                    w_op(ko, 1)[:, q * 256 : q * 256 + 256],
                                    start=ko == 0,
                                    stop=ko == KO - 1,
                                )
                            evict(psq, ot[:, lo : lo + 256], lo, lo + 256)
                            eng = nc.scalar if q == 0 else nc.sync
                            eng.dma_start(y_v[:, mt, lo : lo + 256], ot[:, lo : lo + 256])

    nc.compile()
    return nc


def _get_nc():
    global _NC_CACHE
    if _NC_CACHE is None:
        _NC_CACHE = _build_nc()
    return _NC_CACHE


def kernel(x, W, b, A, B):
    global LAST_RESULT
    x = np.ascontiguousarray(np.asarray(x, dtype=np.float32))
    W = np.asarray(W, dtype=np.float32)
    b = np.asarray(b, dtype=np.float32)
    A = np.asarray(A, dtype=np.float32)
    B = np.asarray(B, dtype=np.float32)
    assert x.shape == (4, 4096, D) and W.shape == (O, D)
    assert b.shape == (O,) and A.shape[1] == D and B.shape[0] == O

    # Fold the LoRA update into the weight: x@W^T + s*(x@A^T)@B^T = x@(W + s*B@A)^T
    Weff = (
        W.astype(np.float64) + SCALING * (B.astype(np.float64) @ A.astype(np.float64))
    ).astype(np.float32)
    WeffT = np.ascontiguousarray(Weff.T).astype(bfloat16)  # [D, O]
    bias_rep = np.ascontiguousarray(np.broadcast_to(b[None, :], (P, O)))

    xr = x.reshape(M_TOTAL, D)
    in_maps = []
    for c in range(N_CORES):
        xTc = np.ascontiguousarray(xr[c * M : (c + 1) * M].T).astype(bfloat16)  # [D, M]
        in_maps.append({"xT": xTc, "wT": WeffT, "bias": bias_rep})

    nc = _get_nc()
    res = run_bass_kernel_spmd(
        nc, in_maps, core_ids=list(range(N_CORES)), trace=TRACE
    )
    LAST_RESULT = res

    out = np.concatenate(
        [res.results[c]["y"].astype(np.float32) for c in range(N_CORES)], axis=0
    )
    return out.reshape(x.shape[0], x.shape[1], O)
